# revision 11
# baseline (speedup 1.0000x reference)
"""3x3 valid conv (single channel) on 8 TRN2 NeuronCores.

Strategy (v2, fp16): the conv is memory-bound and the cost model serializes
all DMA on one 360 GB/s resource, so I/O is fp16 end-to-end (host converts
X -> fp16, output upcast f32 on host; rel err ~1e-3 << 2e-2 gate). Matmuls
run in fp16 (1 cyc/row vs fp32r's effective half rate on HW).

Work split: 4094 output rows = 32 full 126-row strips (4 per core, rows
504i..504i+504) + a 62-row tail strip (rows 4032..4093) column-sharded
1024 cols/core so all cores do equal tensor work (66 col-tile groups each).
Per core the whole fp16 input (8.4 MB) is SBUF-resident: strip tiles load
once up front, the tensor engine then runs one continuous burst (a memset+
dummy-matmul warmup ramps the PE p-state to 2.4 GHz before real work).
Conv per col tile = 3 banded matmuls (dj = 0..2) accumulating in PSUM:
    out[m, c] = sum_dj (B_dj.T @ X_tile[:, c+dj])[m],  B_dj[k, m] = W[k-m, dj]
PSUM drains (bias add + fp16 cast) alternate Scalar/Vector engines; stores
ride the same SP HWDGE ring after all loads are queued.
"""

import sys

sys.path.insert(0, "/opt/trn_rl_repo")

import numpy as np
from concourse import bass, mybir
from concourse.bass_utils import run_bass_kernel_spmd
from concourse.tile import TileContext

F16 = mybir.dt.float16
F32 = mybir.dt.float32

H, WIDTH = 4096, 8192
KH, KW = 3, 3
OH, OW = H - KH + 1, WIDTH - KW + 1   # 4094, 8190
N_CORES = 8
MAIN_RPC = 504                        # 4 strips x 126 output rows per core
N_STRIPS = 4
TAIL_R0 = N_CORES * MAIN_RPC          # 4032; tail rows 4032..4093 (62 rows)
TAIL_ROWS = OH - TAIL_R0              # 62
TAIL_IN_R0 = H - 128                  # 3968: load rows 3968..4095, outputs at m=64..125
TAIL_COLS = 1024                      # output cols per core in the tail strip
N_COL_TILES = 16                      # 15 x 512 + 1 x 510 = 8190
GROUP = 4                             # col tiles staged per output DMA (~516 KB)
N_WARM = 7                            # dummy matmuls to ramp the PE p-state
# strip-0 column chunks: small first loads so real matmuls can start the
# moment the warmup ends (col tile t needs cols [512t, 512t+514)). The
# [4104, 8192) chunk is queued BEFORE [2056, 4104): col tiles 4-7 (which
# need the latter) run ~4us after tile 8 's data would otherwise arrive.
S0_CHUNKS = [(0, 516), (516, 1032), (1032, 2056), (4104, WIDTH), (2056, 4104)]


def _split_multi_waits(nc, max_waits=1):
    # This container's walrus rejects >1 sync-wait command per instruction
    # (CoreV3 setupSyncWait). Tile attaches one wait per producing logical
    # processor to a single instruction; hoist the excess onto same-engine
    # Drain carriers inserted immediately before it.
    for fn in nc.m.functions:
        for bb in fn.blocks:
            out = []
            changed = False
            for inst in bb.instructions:
                si = inst.sync_info
                waits = list(si.on_wait) if si and si.on_wait else []
                if len(waits) > max_waits:
                    rest = waits[max_waits:]
                    for j in range(0, len(rest), max_waits):
                        carrier = mybir.InstDrain(
                            name=nc.get_next_instruction_name(), ins=[], outs=[]
                        )
                        carrier.engine = inst.engine
                        carrier.sync_info = mybir.SyncInfo(
                            on_wait=rest[j : j + max_waits], on_update=[]
                        )
                        out.append(carrier)
                    si.on_wait = waits[:max_waits]
                    changed = True
                out.append(inst)
            if changed:
                bb.instructions = out


def _build(split_waits=True):
    nc = bass.Bass()
    x = nc.declare_dram_parameter("x", [506, WIDTH], F16, isOutput=False)
    xt = nc.declare_dram_parameter("xt", [128, TAIL_COLS + 2], F16, isOutput=False)
    bands = nc.declare_dram_parameter("bands", [128, 6 * 128], F16, isOutput=False)
    bias = nc.declare_dram_parameter("bias", [128, 1], F32, isOutput=False)
    y = nc.declare_dram_parameter("y", [MAIN_RPC, OW], F16, isOutput=True)
    yt = nc.declare_dram_parameter("yt", [TAIL_ROWS, TAIL_COLS], F16, isOutput=True)

    ident = mybir.ActivationFunctionType.Identity

    with TileContext(nc) as tc:
        with (
            tc.tile_pool(name="const", bufs=1) as cpool,
            tc.tile_pool(name="xin", bufs=4) as xpool,
            tc.tile_pool(name="xtail", bufs=1) as tpool,
            tc.tile_pool(name="stage", bufs=10) as spool,
            tc.tile_pool(name="psum", bufs=6, space="PSUM") as ppool,
            tc.tile_pool(name="warmp", bufs=1, space="PSUM") as wpool,
        ):
            # PE p-state warmup: memset a dummy tile, then back-to-back dummy
            # matmuls so the PE clock is ramping while the first loads are in
            # flight and real work starts with no PE gap.
            warm = cpool.tile([128, 512], F16)
            nc.gpsimd.memset(warm[:], 0.0)
            wps = wpool.tile([128, 512], F32, tag="wps")
            for _ in range(N_WARM):
                nc.tensor.matmul(
                    wps[:126, :512], warm[:, :126], warm[:, :512],
                    start=True, stop=True,
                )

            band_t = cpool.tile([128, 6 * 128], F16)
            nc.gpsimd.dma_start(out=band_t[:], in_=bands[:])
            bias_t = cpool.tile([128, 1], F32)
            nc.gpsimd.dma_start(out=bias_t[:], in_=bias[:])

            # All input loads queue up front on the SP ring (whole input is
            # SBUF-resident). Strip 0 in quarters so compute starts early.
            xts = []
            for s in range(N_STRIPS):
                xtile = xpool.tile([128, WIDTH], F16, tag="xs")
                r0 = 126 * s
                if s == 0:
                    for c0, c1 in S0_CHUNKS:
                        nc.sync.dma_start(
                            out=xtile[:, c0:c1],
                            in_=x[r0 : r0 + 128, c0:c1],
                        )
                else:
                    nc.sync.dma_start(
                        out=xtile[:, 0:4096], in_=x[r0 : r0 + 128, 0:4096]
                    )
                    nc.sync.dma_start(
                        out=xtile[:, 4096:WIDTH], in_=x[r0 : r0 + 128, 4096:WIDTH]
                    )
                xts.append(xtile)
            xtail_t = tpool.tile([128, TAIL_COLS + 2], F16)
            nc.sync.dma_start(out=xtail_t[:], in_=xt[:])

            def col_tile(s, t, stage, drain_dve):
                c0 = t * 512
                n = 512 if t < N_COL_TILES - 1 else 510
                ps = ppool.tile([128, 512], F32, tag="ps", name=f"ps_{s}_{t}")
                for dj in range(KW):
                    nc.tensor.matmul(
                        ps[:126, :n],
                        band_t[:, dj * 128 : dj * 128 + 126],
                        xts[s][:, c0 + dj : c0 + dj + n],
                        start=(dj == 0),
                        stop=(dj == KW - 1),
                    )
                j = t % GROUP
                if drain_dve:
                    nc.vector.tensor_scalar_add(
                        stage[:126, j * 512 : j * 512 + n],
                        ps[:126, :n],
                        bias_t[:126, :],
                    )
                else:
                    nc.scalar.activation(
                        stage[:126, j * 512 : j * 512 + n],
                        ps[:126, :n],
                        ident,
                        bias=bias_t[:126, :],
                        scale=1.0,
                    )

            def tail_tile(j):
                # 62 rows x 512 cols; band cols 0..61 (input tile row k =
                # global row 3968+k; band row k -> out m = k-64).
                c0 = j * 512
                ps = ppool.tile([128, 512], F32, tag="ps", name=f"ps_t{j}")
                for dj in range(KW):
                    nc.tensor.matmul(
                        ps[:TAIL_ROWS, :512],
                        band_t[:, 384 + dj * 128 : 384 + dj * 128 + TAIL_ROWS],
                        xtail_t[:, c0 + dj : c0 + dj + 512],
                        start=(dj == 0),
                        stop=(dj == KW - 1),
                    )
                if j == 0:
                    nc.scalar.activation(
                        stage_t[:TAIL_ROWS, c0 : c0 + 512],
                        ps[:TAIL_ROWS, :512],
                        ident,
                        bias=bias_t[:TAIL_ROWS, :],
                        scale=1.0,
                    )
                else:
                    nc.vector.tensor_scalar_add(
                        stage_t[:TAIL_ROWS, c0 : c0 + 512],
                        ps[:TAIL_ROWS, :512],
                        bias_t[:TAIL_ROWS, :],
                    )

            # Main strips: 4 x 16 col tiles, 3 banded matmuls each. The tail
            # strip (62 rows x 1024 cols) is computed before strip 3's last
            # group, and that group's store is split in two, so the final
            # DMA after the last drain is small.
            stage_t = spool.tile([128, TAIL_COLS], F16, tag="stail")
            for s in range(N_STRIPS):
                y0 = 126 * s
                last = s == N_STRIPS - 1
                for g in range(N_COL_TILES // GROUP):
                    gw = 2048 if g < 3 else 2046
                    stage = spool.tile(
                        [128, 2048], F16, tag="stage", name=f"stage_{s}_{g}"
                    )
                    if last and g == 3:
                        tail_tile(0)
                        tail_tile(1)
                        nc.sync.dma_start(
                            out=yt[:, :], in_=stage_t[:TAIL_ROWS, :TAIL_COLS]
                        )
                    for j in range(GROUP):
                        col_tile(s, g * GROUP + j, stage, drain_dve=(j % 2 == 1))
                        if last and g == 3 and j == 1:
                            nc.sync.dma_start(
                                out=y[y0 : y0 + 126, 6144:7168],
                                in_=stage[:126, 0:1024],
                            )
                    if last and g == 3:
                        nc.sync.dma_start(
                            out=y[y0 : y0 + 126, 7168:8190],
                            in_=stage[:126, 1024:2046],
                        )
                    else:
                        nc.sync.dma_start(
                            out=y[y0 : y0 + 126, g * 2048 : g * 2048 + gw],
                            in_=stage[:126, :gw],
                        )

    if split_waits:
        _split_multi_waits(nc)
    return nc


_NC_CACHE = None


def _get_nc():
    global _NC_CACHE
    if _NC_CACHE is None:
        _NC_CACHE = _build()
    return _NC_CACHE


def _make_host_inputs(X, W, b):
    X16 = np.ascontiguousarray(np.asarray(X, dtype=np.float32)).astype(np.float16)
    W16 = np.asarray(W, dtype=np.float32).astype(np.float16)
    b = np.asarray(b, dtype=np.float32)

    bands = np.zeros((128, 6 * 128), dtype=np.float16)
    mm = np.arange(126)
    mt = np.arange(TAIL_ROWS)
    for dj in range(KW):
        for dk in range(KH):
            # main band: B_dj[m+dk, m] = W[dk, dj] for output rows m=0..125
            bands[mm + dk, dj * 128 + mm] = W16[dk, dj]
            # tail band: out m=0..61 <-> input tile row 64+m+dk
            bands[64 + mt + dk, 384 + dj * 128 + mt] = W16[dk, dj]
    bias = np.full((128, 1), float(b[0]), dtype=np.float32)

    in_maps = []
    for i in range(N_CORES):
        r0 = i * MAIN_RPC
        shard = X16[r0 : r0 + 506]
        c0 = i * TAIL_COLS
        cw = min(TAIL_COLS + 2, WIDTH - c0)
        tail = X16[TAIL_IN_R0:, c0 : c0 + cw]
        if cw < TAIL_COLS + 2:
            tail = np.pad(tail, ((0, 0), (0, TAIL_COLS + 2 - cw)))
        in_maps.append(
            {"x": shard, "xt": np.ascontiguousarray(tail), "bands": bands, "bias": bias}
        )
    return in_maps


def _assemble(results):
    out = np.empty((OH, OW), dtype=np.float32)
    for i in range(N_CORES):
        r0 = i * MAIN_RPC
        out[r0 : r0 + MAIN_RPC] = results[i]["y"].astype(np.float32)
        c0 = i * TAIL_COLS
        w = min(TAIL_COLS, OW - c0)
        out[TAIL_R0:OH, c0 : c0 + w] = results[i]["yt"][:, :w].astype(np.float32)
    return out


def run(X, W, b, trace=False):
    nc = _get_nc()
    in_maps = _make_host_inputs(X, W, b)
    res = run_bass_kernel_spmd(nc, in_maps, list(range(N_CORES)), trace=trace)
    return _assemble(res.results), res


def kernel(X, W, b):
    out, _ = run(X, W, b)
    return out


# revision 16
# speedup vs baseline: 1.0353x; 1.0353x over previous
"""3x3 valid conv (single channel) on 8 TRN2 NeuronCores.

Strategy (v2, fp16): the conv is memory-bound and the cost model serializes
all DMA on one 360 GB/s resource, so I/O is fp16 end-to-end (host converts
X -> fp16, output upcast f32 on host; rel err ~1e-3 << 2e-2 gate). Matmuls
run in fp16 (1 cyc/row vs fp32r's effective half rate on HW).

Work split: 4094 output rows = 32 full 126-row strips (4 per core, rows
504i..504i+504) + a 62-row tail strip (rows 4032..4093) column-sharded
1024 cols/core so all cores do equal tensor work (66 col-tile groups each).
Per core the whole fp16 input (8.4 MB) is SBUF-resident: strip tiles load
once up front, the tensor engine then runs one continuous burst (a memset+
dummy-matmul warmup ramps the PE p-state to 2.4 GHz before real work).
Conv per col tile = 3 banded matmuls (dj = 0..2) accumulating in PSUM:
    out[m, c] = sum_dj (B_dj.T @ X_tile[:, c+dj])[m],  B_dj[k, m] = W[k-m, dj]
PSUM drains (bias add + fp16 cast) alternate Scalar/Vector engines; stores
ride the same SP HWDGE ring after all loads are queued.
"""

import sys

sys.path.insert(0, "/opt/trn_rl_repo")

import numpy as np
from concourse import bass, mybir
from concourse.bass_utils import run_bass_kernel_spmd
from concourse.tile import TileContext

F16 = mybir.dt.float16
F32 = mybir.dt.float32

H, WIDTH = 4096, 8192
KH, KW = 3, 3
OH, OW = H - KH + 1, WIDTH - KW + 1   # 4094, 8190
N_CORES = 8
MAIN_RPC = 504                        # 4 strips x 126 output rows per core
N_STRIPS = 4
TAIL_R0 = N_CORES * MAIN_RPC          # 4032; tail rows 4032..4093 (62 rows)
TAIL_ROWS = OH - TAIL_R0              # 62
TAIL_IN_R0 = H - 128                  # 3968: load rows 3968..4095, outputs at m=64..125
TAIL_COLS = 1024                      # output cols per core in the tail strip
N_COL_TILES = 16                      # 15 x 512 + 1 x 510 = 8190
GROUP = 4                             # col tiles staged per output DMA (~516 KB)
N_WARM = 6                            # plain dummy matmuls to ramp the PE p-state
# strip-0 column chunks: small first loads so real matmuls can start the
# moment the warmup ends (col tile t needs cols [512t, 512t+514))
S0_CHUNKS = [(0, 516), (516, 1032), (1032, 2056), (2056, 4104), (4104, 6152),
             (6152, WIDTH)]


def _split_multi_waits(nc, max_waits=1):
    # This container's walrus rejects >1 sync-wait command per instruction
    # (CoreV3 setupSyncWait). Tile attaches one wait per producing logical
    # processor to a single instruction; hoist the excess onto same-engine
    # Drain carriers inserted immediately before it.
    for fn in nc.m.functions:
        for bb in fn.blocks:
            out = []
            changed = False
            for inst in bb.instructions:
                si = inst.sync_info
                waits = list(si.on_wait) if si and si.on_wait else []
                if len(waits) > max_waits:
                    rest = waits[max_waits:]
                    for j in range(0, len(rest), max_waits):
                        carrier = mybir.InstDrain(
                            name=nc.get_next_instruction_name(), ins=[], outs=[]
                        )
                        carrier.engine = inst.engine
                        carrier.sync_info = mybir.SyncInfo(
                            on_wait=rest[j : j + max_waits], on_update=[]
                        )
                        out.append(carrier)
                    si.on_wait = waits[:max_waits]
                    changed = True
                out.append(inst)
            if changed:
                bb.instructions = out


def _build(split_waits=True):
    nc = bass.Bass()
    x = nc.declare_dram_parameter("x", [506, WIDTH], F16, isOutput=False)
    xt = nc.declare_dram_parameter("xt", [128, TAIL_COLS + 2], F16, isOutput=False)
    bands = nc.declare_dram_parameter("bands", [128, 6 * 128], F16, isOutput=False)
    bias = nc.declare_dram_parameter("bias", [128, 1], F32, isOutput=False)
    y = nc.declare_dram_parameter("y", [MAIN_RPC, OW], F16, isOutput=True)
    yt = nc.declare_dram_parameter("yt", [TAIL_ROWS, TAIL_COLS], F16, isOutput=True)

    ident = mybir.ActivationFunctionType.Identity

    with TileContext(nc) as tc:
        with (
            tc.tile_pool(name="const", bufs=1) as cpool,
            tc.tile_pool(name="xin", bufs=4) as xpool,
            tc.tile_pool(name="xtail", bufs=1) as tpool,
            tc.tile_pool(name="stage", bufs=10) as spool,
            tc.tile_pool(name="psum", bufs=6, space="PSUM") as ppool,
            tc.tile_pool(name="warmp", bufs=1, space="PSUM") as wpool,
        ):
            # PE p-state warmup: memset a dummy tile, then back-to-back dummy
            # matmuls so the PE clock is ramping while the first loads are in
            # flight and real work starts with no PE gap.
            warm = cpool.tile([128, 512], F16)
            nc.gpsimd.memset(warm[:], 0.0)
            wps = wpool.tile([128, 512], F32, tag="wps")

            band_t = cpool.tile([128, 6 * 128], F16)
            nc.gpsimd.dma_start(out=band_t[:], in_=bands[:])
            bias_t = cpool.tile([128, 1], F32)
            nc.gpsimd.dma_start(out=bias_t[:], in_=bias[:])

            for _ in range(N_WARM):
                nc.tensor.matmul(
                    wps[:126, :512], warm[:, :126], warm[:, :512],
                    start=True, stop=True,
                )

            def touch(ap):
                # Absorb a DMA-completion semaphore into a ~7ns 16-col dummy
                # matmul, so the first real matmul reading that data doesn't
                # carry a fresh sem wait (multi-wait matmuls open a PE-engine
                # gap that knocks the p-state down).
                nc.tensor.matmul(
                    wps[:126, :16], band_t[:, 0:126], ap,
                    start=True, stop=True,
                )

            # All input loads queue up front on the SP ring (whole input is
            # SBUF-resident). Strip 0 in quarters so compute starts early.
            xts = []
            for s in range(N_STRIPS):
                xtile = xpool.tile([128, WIDTH], F16, tag="xs")
                r0 = 126 * s
                if s == 0:
                    for c0, c1 in S0_CHUNKS:
                        nc.sync.dma_start(
                            out=xtile[:, c0:c1],
                            in_=x[r0 : r0 + 128, c0:c1],
                        )
                else:
                    nc.sync.dma_start(
                        out=xtile[:, 0:4096], in_=x[r0 : r0 + 128, 0:4096]
                    )
                    nc.sync.dma_start(
                        out=xtile[:, 4096:WIDTH], in_=x[r0 : r0 + 128, 4096:WIDTH]
                    )
                xts.append(xtile)
            xtail_t = tpool.tile([128, TAIL_COLS + 2], F16)
            nc.sync.dma_start(out=xtail_t[:], in_=xt[:])

            # Finish the warmup by absorbing the band + first-chunk sems.
            touch(warm[:, 0:16])            # stationary band_t: absorbs band
            touch(xts[0][:, 0:16])          # chunk A
            touch(xts[0][:, 516:532])       # chunk B

            # (strip, tile) -> moving-operand slivers whose DMA sems are
            # absorbed right after that tile, well after each sem fires but
            # before the first real matmul needs the data.
            touches = {
                (0, 0): [xts[0][:, 1032:1048]],    # C
                (0, 3): [xts[0][:, 2056:2072]],    # D
                (0, 5): [xts[0][:, 4104:4120]],    # E1
                (0, 9): [xts[0][:, 6152:6168]],    # E2
                (0, 12): [xts[1][:, 0:16]],        # S1a
                (0, 14): [xts[1][:, 4096:4112]],   # S1b
                (1, 4): [xts[2][:, 0:16]],         # S2a
                (1, 6): [xts[2][:, 4096:4112]],    # S2b
                (2, 4): [xts[3][:, 0:16]],         # S3a
                (2, 6): [xts[3][:, 4096:4112]],    # S3b
                (3, 4): [xtail_t[:, 0:16]],        # tail
            }

            def col_tile(s, t, stage, drain_dve):
                c0 = t * 512
                n = 512 if t < N_COL_TILES - 1 else 510
                ps = ppool.tile([128, 512], F32, tag="ps", name=f"ps_{s}_{t}")
                for dj in range(KW):
                    nc.tensor.matmul(
                        ps[:126, :n],
                        band_t[:, dj * 128 : dj * 128 + 126],
                        xts[s][:, c0 + dj : c0 + dj + n],
                        start=(dj == 0),
                        stop=(dj == KW - 1),
                    )
                j = t % GROUP
                if drain_dve:
                    nc.vector.tensor_scalar_add(
                        stage[:126, j * 512 : j * 512 + n],
                        ps[:126, :n],
                        bias_t[:126, :],
                    )
                else:
                    nc.scalar.activation(
                        stage[:126, j * 512 : j * 512 + n],
                        ps[:126, :n],
                        ident,
                        bias=bias_t[:126, :],
                        scale=1.0,
                    )

            def tail_tile(j):
                # 62 rows x 512 cols; band cols 0..61 (input tile row k =
                # global row 3968+k; band row k -> out m = k-64).
                c0 = j * 512
                ps = ppool.tile([128, 512], F32, tag="ps", name=f"ps_t{j}")
                for dj in range(KW):
                    nc.tensor.matmul(
                        ps[:TAIL_ROWS, :512],
                        band_t[:, 384 + dj * 128 : 384 + dj * 128 + TAIL_ROWS],
                        xtail_t[:, c0 + dj : c0 + dj + 512],
                        start=(dj == 0),
                        stop=(dj == KW - 1),
                    )
                if j == 0:
                    nc.scalar.activation(
                        stage_t[:TAIL_ROWS, c0 : c0 + 512],
                        ps[:TAIL_ROWS, :512],
                        ident,
                        bias=bias_t[:TAIL_ROWS, :],
                        scale=1.0,
                    )
                else:
                    nc.vector.tensor_scalar_add(
                        stage_t[:TAIL_ROWS, c0 : c0 + 512],
                        ps[:TAIL_ROWS, :512],
                        bias_t[:TAIL_ROWS, :],
                    )

            # Main strips: 4 x 16 col tiles, 3 banded matmuls each. The tail
            # strip (62 rows x 1024 cols) is computed before strip 3's last
            # group, and that group's store is split in two, so the final
            # DMA after the last drain is small.
            stage_t = spool.tile([128, TAIL_COLS], F16, tag="stail")
            for s in range(N_STRIPS):
                y0 = 126 * s
                last = s == N_STRIPS - 1
                for g in range(N_COL_TILES // GROUP):
                    gw = 2048 if g < 3 else 2046
                    stage = spool.tile(
                        [128, 2048], F16, tag="stage", name=f"stage_{s}_{g}"
                    )
                    if last and g == 3:
                        tail_tile(0)
                        tail_tile(1)
                        nc.sync.dma_start(
                            out=yt[:, :], in_=stage_t[:TAIL_ROWS, :TAIL_COLS]
                        )
                    for j in range(GROUP):
                        t = g * GROUP + j
                        col_tile(s, t, stage, drain_dve=(j % 2 == 1))
                        for ap in touches.get((s, t), ()):
                            touch(ap)
                        if last and g == 3 and j == 1:
                            nc.sync.dma_start(
                                out=y[y0 : y0 + 126, 6144:7168],
                                in_=stage[:126, 0:1024],
                            )
                    if last and g == 3:
                        nc.sync.dma_start(
                            out=y[y0 : y0 + 126, 7168:8190],
                            in_=stage[:126, 1024:2046],
                        )
                    else:
                        nc.sync.dma_start(
                            out=y[y0 : y0 + 126, g * 2048 : g * 2048 + gw],
                            in_=stage[:126, :gw],
                        )

    if split_waits:
        _split_multi_waits(nc)
    return nc


_NC_CACHE = None


def _get_nc():
    global _NC_CACHE
    if _NC_CACHE is None:
        _NC_CACHE = _build()
    return _NC_CACHE


def _make_host_inputs(X, W, b):
    X16 = np.ascontiguousarray(np.asarray(X, dtype=np.float32)).astype(np.float16)
    W16 = np.asarray(W, dtype=np.float32).astype(np.float16)
    b = np.asarray(b, dtype=np.float32)

    bands = np.zeros((128, 6 * 128), dtype=np.float16)
    mm = np.arange(126)
    mt = np.arange(TAIL_ROWS)
    for dj in range(KW):
        for dk in range(KH):
            # main band: B_dj[m+dk, m] = W[dk, dj] for output rows m=0..125
            bands[mm + dk, dj * 128 + mm] = W16[dk, dj]
            # tail band: out m=0..61 <-> input tile row 64+m+dk
            bands[64 + mt + dk, 384 + dj * 128 + mt] = W16[dk, dj]
    bias = np.full((128, 1), float(b[0]), dtype=np.float32)

    in_maps = []
    for i in range(N_CORES):
        r0 = i * MAIN_RPC
        shard = X16[r0 : r0 + 506]
        c0 = i * TAIL_COLS
        cw = min(TAIL_COLS + 2, WIDTH - c0)
        tail = X16[TAIL_IN_R0:, c0 : c0 + cw]
        if cw < TAIL_COLS + 2:
            tail = np.pad(tail, ((0, 0), (0, TAIL_COLS + 2 - cw)))
        in_maps.append(
            {"x": shard, "xt": np.ascontiguousarray(tail), "bands": bands, "bias": bias}
        )
    return in_maps


def _assemble(results):
    out = np.empty((OH, OW), dtype=np.float32)
    for i in range(N_CORES):
        r0 = i * MAIN_RPC
        out[r0 : r0 + MAIN_RPC] = results[i]["y"].astype(np.float32)
        c0 = i * TAIL_COLS
        w = min(TAIL_COLS, OW - c0)
        out[TAIL_R0:OH, c0 : c0 + w] = results[i]["yt"][:, :w].astype(np.float32)
    return out


def run(X, W, b, trace=False):
    nc = _get_nc()
    in_maps = _make_host_inputs(X, W, b)
    res = run_bass_kernel_spmd(nc, in_maps, list(range(N_CORES)), trace=trace)
    return _assemble(res.results), res


def kernel(X, W, b):
    out, _ = run(X, W, b)
    return out


# revision 21
# speedup vs baseline: 1.0645x; 1.0282x over previous
"""3x3 valid conv (single channel) on 8 TRN2 NeuronCores.

Strategy (v2, fp16): the conv is memory-bound and the cost model serializes
all DMA on one 360 GB/s resource, so I/O is fp16 end-to-end (host converts
X -> fp16, output upcast f32 on host; rel err ~1e-3 << 2e-2 gate). Matmuls
run in fp16 (1 cyc/row vs fp32r's effective half rate on HW).

Work split: 4094 output rows = 32 full 126-row strips (4 per core, rows
504i..504i+504) + a 62-row tail strip (rows 4032..4093) column-sharded
1024 cols/core so all cores do equal tensor work (66 col-tile groups each).
Per core the whole fp16 input (8.4 MB) is SBUF-resident: strip tiles load
once up front, the tensor engine then runs one continuous burst (a memset+
dummy-matmul warmup ramps the PE p-state to 2.4 GHz before real work).
Conv per col tile = 3 banded matmuls (dj = 0..2) accumulating in PSUM:
    out[m, c] = sum_dj (B_dj.T @ X_tile[:, c+dj])[m],  B_dj[k, m] = W[k-m, dj]
PSUM drains (bias add + fp16 cast) alternate Scalar/Vector engines; stores
ride the same SP HWDGE ring after all loads are queued.
"""

import sys

sys.path.insert(0, "/opt/trn_rl_repo")

import numpy as np
from concourse import bass, mybir
from concourse.bass_utils import run_bass_kernel_spmd
from concourse.tile import TileContext

F16 = mybir.dt.float16
F32 = mybir.dt.float32

H, WIDTH = 4096, 8192
KH, KW = 3, 3
OH, OW = H - KH + 1, WIDTH - KW + 1   # 4094, 8190
N_CORES = 8
MAIN_RPC = 504                        # 4 strips x 126 output rows per core
N_STRIPS = 4
TAIL_R0 = N_CORES * MAIN_RPC          # 4032; tail rows 4032..4093 (62 rows)
TAIL_ROWS = OH - TAIL_R0              # 62
TAIL_IN_R0 = H - 128                  # 3968: load rows 3968..4095, outputs at m=64..125
TAIL_COLS = 1024                      # output cols per core in the tail strip
N_COL_TILES = 16                      # 15 x 512 + 1 x 510 = 8190
GROUP = 4                             # col tiles staged per output DMA (~516 KB)
N_WARM = 7                            # plain dummy matmuls to ramp the PE p-state
# strip-0 column chunks: small first loads so real matmuls can start the
# moment the warmup ends (col tile t needs cols [512t, 512t+514))
S0_CHUNKS = [(0, 516), (516, 1032), (1032, 2056), (2056, 3080), (3080, 4104),
             (4104, 6152), (6152, WIDTH)]


def _split_multi_waits(nc, max_waits=1):
    # This container's walrus rejects >1 sync-wait command per instruction
    # (CoreV3 setupSyncWait). Tile attaches one wait per producing logical
    # processor to a single instruction; hoist the excess onto same-engine
    # Drain carriers inserted immediately before it.
    for fn in nc.m.functions:
        for bb in fn.blocks:
            out = []
            changed = False
            for inst in bb.instructions:
                si = inst.sync_info
                waits = list(si.on_wait) if si and si.on_wait else []
                if len(waits) > max_waits:
                    rest = waits[max_waits:]
                    for j in range(0, len(rest), max_waits):
                        carrier = mybir.InstDrain(
                            name=nc.get_next_instruction_name(), ins=[], outs=[]
                        )
                        carrier.engine = inst.engine
                        carrier.sync_info = mybir.SyncInfo(
                            on_wait=rest[j : j + max_waits], on_update=[]
                        )
                        out.append(carrier)
                    si.on_wait = waits[:max_waits]
                    changed = True
                out.append(inst)
            if changed:
                bb.instructions = out


def _build(split_waits=True):
    nc = bass.Bass()
    x = nc.declare_dram_parameter("x", [506, WIDTH], F16, isOutput=False)
    xt = nc.declare_dram_parameter("xt", [128, TAIL_COLS + 2], F16, isOutput=False)
    bands = nc.declare_dram_parameter("bands", [128, 6 * 128], F16, isOutput=False)
    bias = nc.declare_dram_parameter("bias", [128, 1], F32, isOutput=False)
    y = nc.declare_dram_parameter("y", [MAIN_RPC, OW], F16, isOutput=True)
    yt = nc.declare_dram_parameter("yt", [TAIL_ROWS, TAIL_COLS], F16, isOutput=True)

    ident = mybir.ActivationFunctionType.Identity

    with TileContext(nc) as tc:
        with (
            tc.tile_pool(name="const", bufs=1) as cpool,
            tc.tile_pool(name="xin", bufs=4) as xpool,
            tc.tile_pool(name="xtail", bufs=1) as tpool,
            tc.tile_pool(name="stage", bufs=10) as spool,
            tc.tile_pool(name="psum", bufs=6, space="PSUM") as ppool,
            tc.tile_pool(name="warmp", bufs=1, space="PSUM") as wpool,
        ):
            # PE p-state warmup: back-to-back dummy matmuls ramp the PE clock
            # while the first loads are in flight, so real work starts with
            # no PE gap. band + bias ride the HWDGE sync ring ahead of the x
            # chunks — the SWDGE path delivered the band ~2us too late and
            # stalled the first real matmul.
            warm = cpool.tile([128, 512], F16)
            nc.gpsimd.memset(warm[:], 0.0)
            wps = wpool.tile([128, 512], F32, tag="wps")

            band_t = cpool.tile([128, 6 * 128], F16)
            nc.sync.dma_start(out=band_t[:], in_=bands[:])
            bias_t = cpool.tile([128, 1], F32)
            nc.sync.dma_start(out=bias_t[:], in_=bias[:])

            for _ in range(N_WARM):
                nc.tensor.matmul(
                    wps[:126, :512], warm[:, :126], warm[:, :512],
                    start=True, stop=True,
                )

            def touch(ap):
                # Absorb a DMA-completion semaphore into a ~7ns 16-col dummy
                # matmul, so the first real matmul reading that data doesn't
                # carry a fresh sem wait (multi-wait matmuls open a PE-engine
                # gap that knocks the p-state down).
                nc.tensor.matmul(
                    wps[:126, :16], band_t[:, 0:126], ap,
                    start=True, stop=True,
                )

            # All input loads queue up front on the SP ring (whole input is
            # SBUF-resident). Strip 0 in quarters so compute starts early.
            xts = []
            for s in range(N_STRIPS):
                xtile = xpool.tile([128, WIDTH], F16, tag="xs")
                r0 = 126 * s
                if s == 0:
                    for c0, c1 in S0_CHUNKS:
                        nc.sync.dma_start(
                            out=xtile[:, c0:c1],
                            in_=x[r0 : r0 + 128, c0:c1],
                        )
                else:
                    nc.sync.dma_start(
                        out=xtile[:, 0:4096], in_=x[r0 : r0 + 128, 0:4096]
                    )
                    nc.sync.dma_start(
                        out=xtile[:, 4096:WIDTH], in_=x[r0 : r0 + 128, 4096:WIDTH]
                    )
                xts.append(xtile)
            xtail_t = tpool.tile([128, TAIL_COLS + 2], F16)
            nc.sync.dma_start(out=xtail_t[:], in_=xt[:])

            # Finish the warmup by absorbing the band + first-chunk sems.
            touch(warm[:, 0:16])            # stationary band_t: absorbs band
            touch(xts[0][:, 0:16])          # chunk A
            touch(xts[0][:, 516:532])       # chunk B

            # (strip, tile) -> moving-operand slivers whose DMA sems are
            # absorbed right after that tile, well after each sem fires but
            # before the first real matmul needs the data.
            touches = {
                (0, 0): [xts[0][:, 1032:1048]],    # C
                (0, 3): [xts[0][:, 2056:2072]],    # D1
                (0, 4): [xts[0][:, 3080:3096]],    # D2
                (0, 6): [xts[0][:, 4104:4120]],    # E1
                (0, 10): [xts[0][:, 6152:6168]],   # E2
                (0, 12): [xts[1][:, 0:16]],        # S1a
                (0, 14): [xts[1][:, 4096:4112]],   # S1b
                (1, 4): [xts[2][:, 0:16]],         # S2a
                (1, 6): [xts[2][:, 4096:4112]],    # S2b
                (2, 4): [xts[3][:, 0:16]],         # S3a
                (2, 6): [xts[3][:, 4096:4112]],    # S3b
                (3, 4): [xtail_t[:, 0:16]],        # tail
            }

            def col_tile(s, t, stage, drain_dve):
                c0 = t * 512
                n = 512 if t < N_COL_TILES - 1 else 510
                ps = ppool.tile([128, 512], F32, tag="ps", name=f"ps_{s}_{t}")
                for dj in range(KW):
                    nc.tensor.matmul(
                        ps[:126, :n],
                        band_t[:, dj * 128 : dj * 128 + 126],
                        xts[s][:, c0 + dj : c0 + dj + n],
                        start=(dj == 0),
                        stop=(dj == KW - 1),
                    )
                j = t % GROUP
                if drain_dve:
                    nc.vector.tensor_scalar_add(
                        stage[:126, j * 512 : j * 512 + n],
                        ps[:126, :n],
                        bias_t[:126, :],
                    )
                else:
                    nc.scalar.activation(
                        stage[:126, j * 512 : j * 512 + n],
                        ps[:126, :n],
                        ident,
                        bias=bias_t[:126, :],
                        scale=1.0,
                    )

            def tail_tile(j):
                # 62 rows x 512 cols; band cols 0..61 (input tile row k =
                # global row 3968+k; band row k -> out m = k-64).
                c0 = j * 512
                ps = ppool.tile([128, 512], F32, tag="ps", name=f"ps_t{j}")
                for dj in range(KW):
                    nc.tensor.matmul(
                        ps[:TAIL_ROWS, :512],
                        band_t[:, 384 + dj * 128 : 384 + dj * 128 + TAIL_ROWS],
                        xtail_t[:, c0 + dj : c0 + dj + 512],
                        start=(dj == 0),
                        stop=(dj == KW - 1),
                    )
                if j == 0:
                    nc.scalar.activation(
                        stage_t[:TAIL_ROWS, c0 : c0 + 512],
                        ps[:TAIL_ROWS, :512],
                        ident,
                        bias=bias_t[:TAIL_ROWS, :],
                        scale=1.0,
                    )
                else:
                    nc.vector.tensor_scalar_add(
                        stage_t[:TAIL_ROWS, c0 : c0 + 512],
                        ps[:TAIL_ROWS, :512],
                        bias_t[:TAIL_ROWS, :],
                    )

            # Main strips: 4 x 16 col tiles, 3 banded matmuls each. The tail
            # strip (62 rows x 1024 cols) is computed before strip 3's last
            # group, and that group's store is split in two, so the final
            # DMA after the last drain is small.
            stage_t = spool.tile([128, TAIL_COLS], F16, tag="stail")
            for s in range(N_STRIPS):
                y0 = 126 * s
                last = s == N_STRIPS - 1
                for g in range(N_COL_TILES // GROUP):
                    gw = 2048 if g < 3 else 2046
                    stage = spool.tile(
                        [128, 2048], F16, tag="stage", name=f"stage_{s}_{g}"
                    )
                    if last and g == 3:
                        tail_tile(0)
                        tail_tile(1)
                        nc.sync.dma_start(
                            out=yt[:, :], in_=stage_t[:TAIL_ROWS, :TAIL_COLS]
                        )
                    for j in range(GROUP):
                        t = g * GROUP + j
                        col_tile(s, t, stage, drain_dve=(j % 2 == 1))
                        for ap in touches.get((s, t), ()):
                            touch(ap)
                        if last and g == 3 and j == 1:
                            nc.sync.dma_start(
                                out=y[y0 : y0 + 126, 6144:7168],
                                in_=stage[:126, 0:1024],
                            )
                    if last and g == 3:
                        nc.sync.dma_start(
                            out=y[y0 : y0 + 126, 7168:8190],
                            in_=stage[:126, 1024:2046],
                        )
                    else:
                        nc.sync.dma_start(
                            out=y[y0 : y0 + 126, g * 2048 : g * 2048 + gw],
                            in_=stage[:126, :gw],
                        )

    if split_waits:
        _split_multi_waits(nc)
    return nc


_NC_CACHE = None


def _get_nc():
    global _NC_CACHE
    if _NC_CACHE is None:
        _NC_CACHE = _build()
    return _NC_CACHE


def _make_host_inputs(X, W, b):
    X16 = np.ascontiguousarray(np.asarray(X, dtype=np.float32)).astype(np.float16)
    W16 = np.asarray(W, dtype=np.float32).astype(np.float16)
    b = np.asarray(b, dtype=np.float32)

    bands = np.zeros((128, 6 * 128), dtype=np.float16)
    mm = np.arange(126)
    mt = np.arange(TAIL_ROWS)
    for dj in range(KW):
        for dk in range(KH):
            # main band: B_dj[m+dk, m] = W[dk, dj] for output rows m=0..125
            bands[mm + dk, dj * 128 + mm] = W16[dk, dj]
            # tail band: out m=0..61 <-> input tile row 64+m+dk
            bands[64 + mt + dk, 384 + dj * 128 + mt] = W16[dk, dj]
    bias = np.full((128, 1), float(b[0]), dtype=np.float32)

    in_maps = []
    for i in range(N_CORES):
        r0 = i * MAIN_RPC
        shard = X16[r0 : r0 + 506]
        c0 = i * TAIL_COLS
        cw = min(TAIL_COLS + 2, WIDTH - c0)
        tail = X16[TAIL_IN_R0:, c0 : c0 + cw]
        if cw < TAIL_COLS + 2:
            tail = np.pad(tail, ((0, 0), (0, TAIL_COLS + 2 - cw)))
        in_maps.append(
            {"x": shard, "xt": np.ascontiguousarray(tail), "bands": bands, "bias": bias}
        )
    return in_maps


def _assemble(results):
    out = np.empty((OH, OW), dtype=np.float32)
    for i in range(N_CORES):
        r0 = i * MAIN_RPC
        out[r0 : r0 + MAIN_RPC] = results[i]["y"].astype(np.float32)
        c0 = i * TAIL_COLS
        w = min(TAIL_COLS, OW - c0)
        out[TAIL_R0:OH, c0 : c0 + w] = results[i]["yt"][:, :w].astype(np.float32)
    return out


def run(X, W, b, trace=False):
    nc = _get_nc()
    in_maps = _make_host_inputs(X, W, b)
    res = run_bass_kernel_spmd(nc, in_maps, list(range(N_CORES)), trace=trace)
    return _assemble(res.results), res


def kernel(X, W, b):
    out, _ = run(X, W, b)
    return out
